# revision 10
# baseline (speedup 1.0000x reference)
"""Trainium2 kernel for nn_BpsMlp: KNN min-distance (B=64,N=1024 queries vs
M=4096 basis points) feeding a 4-layer MLP, data-parallel over batch across
8 NeuronCores.

Per core (8 batches = 8192 query rows):
  - distance phase: d2[q,m] accumulated exactly in fp32 PSUM via one
    K=16-augmented bf16 hi/lo matmul (catastrophic-cancellation-free),
    min over m via ScalarE PSUM->SBUF copy + VectorE tensor_tensor_scan
    (min, min) consuming 2 fresh elems/cycle.
  - x = sqrt(max(d2min, 1e-12)) with one Newton refinement step.
  - MLP in fp16 (weights resident in SBUF), h^T layout [hid-tile 128, batch 8].
"""

import sys

sys.path.insert(0, "/opt/trn_rl_repo")

import numpy as np
import ml_dtypes

import concourse.bass as bass
import concourse.mybir as mybir
import concourse.tile as tile
from concourse.bass import ds, ts
from concourse.bass_utils import run_bass_kernel_spmd

BF16 = ml_dtypes.bfloat16
DT = mybir.dt
AF = mybir.ActivationFunctionType
OP = mybir.AluOpType

B, N, M = 64, 1024, 4096
HID, OUT = 2048, 512
NCORES = 8
BPC = B // NCORES            # batches per core
R = BPC * N                  # query rows per core (8192)
QT = R // 128                # q-tiles per core (64)
KAUG = 16                    # augmented contraction dim
MT_H = HID // 128            # hid tiles (16)
KT1 = N // 128               # L1 k-tiles (8)
KT2 = HID // 128             # L2/L3/L4 k-tiles (16)
MT_O = OUT // 128            # out tiles (4)

_cache = {}


def _split_hi_lo(v):
    vh = v.astype(BF16).astype(np.float32)
    vl = (v - vh).astype(BF16).astype(np.float32)
    return vh, vl


def _build_program():
    nc = bass.Bass()

    posT = nc.declare_dram_parameter("posT_aug", [16, R], DT.bfloat16, isOutput=False)
    basisA = nc.declare_dram_parameter("basis_aug", [16, M], DT.bfloat16, isOutput=False)
    w0 = nc.declare_dram_parameter("w0", [128, KT1 * HID], DT.float16, isOutput=False)
    w1 = nc.declare_dram_parameter("w1", [128, KT2 * HID], DT.float16, isOutput=False)
    w2 = nc.declare_dram_parameter("w2", [128, KT2 * HID], DT.float16, isOutput=False)
    w3 = nc.declare_dram_parameter("w3", [128, KT2 * OUT], DT.float16, isOutput=False)
    b0d = nc.declare_dram_parameter("b0t", [128, MT_H], DT.float32, isOutput=False)
    b1d = nc.declare_dram_parameter("b1t", [128, MT_H], DT.float32, isOutput=False)
    b2d = nc.declare_dram_parameter("b2t", [128, MT_H], DT.float32, isOutput=False)
    b3d = nc.declare_dram_parameter("b3t", [128, MT_O], DT.float32, isOutput=False)
    outT = nc.declare_dram_parameter("outT", [MT_O, 128, BPC], DT.float32, isOutput=True)

    with tile.TileContext(nc) as tc:
        with (
            tc.tile_pool(name="const", bufs=1) as const,
            tc.tile_pool(name="psum", bufs=2, space="PSUM") as psum,
            tc.tile_pool(name="drain", bufs=3) as drain,
        ):
            basis_sb = const.tile([16, M], DT.bfloat16)
            nc.sync.dma_start(basis_sb[:], basisA[:])

            w0_sb = const.tile([128, KT1 * HID], DT.float16)
            w1_sb = const.tile([128, KT2 * HID], DT.float16)
            w2_sb = const.tile([128, KT2 * HID], DT.float16)
            w3_sb = const.tile([128, KT2 * OUT], DT.float16)
            for j in range(KT1):
                nc.sync.dma_start(w0_sb[:, ts(j, HID)], w0[:, ts(j, HID)])
            for j in range(KT2):
                nc.sync.dma_start(w1_sb[:, ts(j, HID)], w1[:, ts(j, HID)])
                nc.sync.dma_start(w2_sb[:, ts(j, HID)], w2[:, ts(j, HID)])
                nc.sync.dma_start(w3_sb[:, ts(j, OUT)], w3[:, ts(j, OUT)])
            b0_sb = const.tile([128, MT_H], DT.float32)
            b1_sb = const.tile([128, MT_H], DT.float32)
            b2_sb = const.tile([128, MT_H], DT.float32)
            b3_sb = const.tile([128, MT_O], DT.float32)
            nc.sync.dma_start(b0_sb[:], b0d[:])
            nc.sync.dma_start(b1_sb[:], b1d[:])
            nc.sync.dma_start(b2_sb[:], b2d[:])
            nc.sync.dma_start(b3_sb[:], b3d[:])

            x_sb = const.tile([128, QT], DT.float32)

            # ---- distance phase ----
            # pos-side lhsT streamed in chunks of 8 q-tiles to save SBUF
            for t in range(QT):
                if t % 8 == 0:
                    pos_chunk = drain.tile([16, 1024], DT.bfloat16, tag="posc")
                    nc.sync.dma_start(pos_chunk[:], posT[:, ts(t // 8, 1024)])
                lhsT = pos_chunk[0:KAUG, ts(t % 8, 128)]
                s_prev = None
                for h in range(2):
                    pt = psum.tile([128, 2048], DT.float32, tag="ps")
                    for j in range(4):
                        nc.tensor.matmul(
                            pt[:, ts(j, 512)],
                            lhsT,
                            basis_sb[0:KAUG, ds(h * 2048 + j * 512, 512)],
                        )
                    cp = drain.tile([128, 1024], DT.float16, tag="cp")
                    nc.scalar.copy(cp[:], pt[:, 1024:2048])
                    s = drain.tile([128, 1024], DT.float16, tag="scan")
                    nc.vector.tensor_tensor_scan(
                        s[:],
                        pt[:, 0:1024],
                        cp[:],
                        initial=(1e30 if h == 0 else s_prev[:, 1023:1024]),
                        op0=OP.min,
                        op1=OP.min,
                    )
                    s_prev = s
                nc.vector.tensor_copy(x_sb[:, t : t + 1], s_prev[:, 1023:1024])

            # ---- x = sqrt(max(d2,1e-12)), one Newton step ----
            xc = const.tile([128, QT], DT.float32)
            nc.vector.tensor_scalar_max(xc[:], x_sb[:], 1e-12)
            y0 = const.tile([128, QT], DT.float32)
            nc.scalar.activation(y0[:], xc[:], AF.Sqrt)
            ry = const.tile([128, QT], DT.float32)
            nc.vector.reciprocal(ry[:], y0[:])
            t1 = const.tile([128, QT], DT.float32)
            nc.vector.tensor_mul(t1[:], xc[:], ry[:])
            t2 = const.tile([128, QT], DT.float32)
            nc.vector.tensor_add(t2[:], y0[:], t1[:])
            xbf = const.tile([128, QT], DT.float16)
            nc.vector.tensor_scalar_mul(xbf[:], t2[:], 0.5)

            # ---- MLP (h^T layout: [hid-tile 128, batch 8]) ----
            xg = xbf[:].rearrange("p (b t) -> p t b", t=KT1)

            def layer(in_view, w_sb, b_sb, n_kt, n_mt, act_relu, out_dtype):
                pt = psum.tile([128, n_mt * BPC], DT.float32, tag="ps")
                hout = drain.tile([128, n_mt * BPC], out_dtype, tag="h" + str(n_mt))
                for mt in range(n_mt):
                    for kt in range(n_kt):
                        nc.tensor.matmul(
                            pt[:, ds(mt * BPC, BPC)],
                            w_sb[:, ds(kt * n_mt * 128 + mt * 128, 128)],
                            in_view[:, kt, :],
                            start=(kt == 0),
                            stop=(kt == n_kt - 1),
                        )
                    if act_relu:
                        nc.scalar.activation(
                            hout[:, ds(mt * BPC, BPC)],
                            pt[:, ds(mt * BPC, BPC)],
                            AF.Relu,
                            bias=b_sb[:, mt : mt + 1],
                        )
                    else:
                        nc.scalar.activation(
                            hout[:, ds(mt * BPC, BPC)],
                            pt[:, ds(mt * BPC, BPC)],
                            AF.Identity,
                            bias=b_sb[:, mt : mt + 1],
                        )
                return hout

            h1 = layer(xg, w0_sb, b0_sb, KT1, MT_H, True, DT.float16)
            h1v = h1[:].rearrange("p (t b) -> p t b", b=BPC)
            h2 = layer(h1v, w1_sb, b1_sb, KT2, MT_H, True, DT.float16)
            h2v = h2[:].rearrange("p (t b) -> p t b", b=BPC)
            h3 = layer(h2v, w2_sb, b2_sb, KT2, MT_H, True, DT.float16)
            h3v = h3[:].rearrange("p (t b) -> p t b", b=BPC)
            h4 = layer(h3v, w3_sb, b3_sb, KT2, MT_O, False, DT.float32)

            for mt in range(MT_O):
                nc.sync.dma_start(outT[mt], h4[:, ds(mt * BPC, BPC)])

    _split_multi_waits(nc)
    return nc


def _split_multi_waits(nc, max_waits=1):
    """neuronx-cc walrus rejects instructions with >1 sync wait; hoist extras
    onto nofuse NOPs just before, on the same engine."""
    ctr = 0
    for f in nc.m.functions:
        for bb in f.blocks:
            new_insts = []
            for ins in bb.instructions:
                si = getattr(ins, "sync_info", None)
                if si is not None and si.on_wait and len(si.on_wait) > max_waits:
                    waits = list(si.on_wait)
                    extra, keep = waits[:-max_waits], waits[-max_waits:]
                    for i in range(0, len(extra), max_waits):
                        ctr += 1
                        new_insts.append(
                            mybir.InstNoOp(
                                name=f"waitsplit-{ctr}",
                                engine=ins.engine,
                                sync_info=mybir.SyncInfo(
                                    on_wait=extra[i : i + max_waits], on_update=[]
                                ),
                                bass_nofuse=True,
                            )
                        )
                    si.on_wait = keep
                new_insts.append(ins)
            bb.instructions[:] = new_insts


def _prep_inputs(pos, basis, W0, b0, W1, b1, W2, b2, W3, b3):
    pos = np.asarray(pos, dtype=np.float32)
    basis = np.asarray(basis, dtype=np.float32)

    bh, bl = _split_hi_lo(basis)  # [M,3]
    q2 = (basis * basis).sum(-1)
    q2h, q2l = _split_hi_lo(q2)
    ones_m = np.ones(M, np.float32)
    basis_aug = np.zeros((16, M), np.float32)
    basis_aug[0:3] = bh.T
    basis_aug[3:6] = bh.T
    basis_aug[6:9] = bl.T
    basis_aug[9:12] = bl.T
    basis_aug[12] = ones_m
    basis_aug[13] = ones_m
    basis_aug[14] = q2h
    basis_aug[15] = q2l
    basis_aug = basis_aug.astype(BF16)

    def pos_aug_for_core(c):
        p = pos[c * BPC : (c + 1) * BPC].reshape(R, 3)
        a = -2.0 * p
        ah, al = _split_hi_lo(a)
        p2 = (p * p).sum(-1)
        p2h, p2l = _split_hi_lo(p2)
        ones_r = np.ones(R, np.float32)
        pa = np.zeros((16, R), np.float32)
        pa[0:3] = ah.T
        pa[3:6] = al.T
        pa[6:9] = ah.T
        pa[9:12] = al.T
        pa[12] = p2h
        pa[13] = p2l
        pa[14] = ones_r
        pa[15] = ones_r
        return pa.astype(BF16)

    def pack_w(W, n_kt, n_out):
        return (
            np.asarray(W, np.float32)
            .reshape(n_kt, 128, n_out)
            .transpose(1, 0, 2)
            .reshape(128, n_kt * n_out)
            .astype(np.float16)
        )

    common = {
        "basis_aug": basis_aug,
        "w0": pack_w(W0, KT1, HID),
        "w1": pack_w(W1, KT2, HID),
        "w2": pack_w(W2, KT2, HID),
        "w3": pack_w(W3, KT2, OUT),
        "b0t": np.asarray(b0, np.float32).reshape(MT_H, 128).T.copy(),
        "b1t": np.asarray(b1, np.float32).reshape(MT_H, 128).T.copy(),
        "b2t": np.asarray(b2, np.float32).reshape(MT_H, 128).T.copy(),
        "b3t": np.asarray(b3, np.float32).reshape(MT_O, 128).T.copy(),
    }
    in_maps = []
    for c in range(NCORES):
        m = dict(common)
        m["posT_aug"] = pos_aug_for_core(c)
        in_maps.append(m)
    return in_maps


def kernel(pos, basis, W0, b0, W1, b1, W2, b2, W3, b3, _trace=False):
    if "nc" not in _cache:
        _cache["nc"] = _build_program()
    nc = _cache["nc"]
    in_maps = _prep_inputs(pos, basis, W0, b0, W1, b1, W2, b2, W3, b3)
    res = run_bass_kernel_spmd(nc, in_maps, list(range(NCORES)), trace=_trace)
    _cache["last_result"] = res
    out = np.empty((B, OUT), np.float32)
    for c in range(NCORES):
        o = np.asarray(res.results[c]["outT"])  # [MT_O, 128, BPC]
        out[c * BPC : (c + 1) * BPC] = o.transpose(2, 0, 1).reshape(BPC, OUT)
    return out
